# revision 5
# baseline (speedup 1.0000x reference)
"""Per-task adapter (MoE routing) on 8 TRN2 NeuronCores.

Strategy: expert-parallel. Host routes rows by task_id so core t gets all
rows with task t, each core computes only its own expert's adapter delta
= silu(x @ Wd[t] + bd[t]) @ Wu[t], and the host scatters deltas back,
adding the f32 residual x and bu[t].

Device kernel is raw bacc (no TileContext) with hand-placed semaphores,
fp8-e4m3 I/O (weights pre-scaled by 16 on the host; the 1/16 is folded
into the silu activation scale, and the up output is descaled on host).

Dataflow per core (capacity CAP=544 = 512 + 32 tail rows):
  in:   x quarters on sync HWDGE, wd on scalar HWDGE, bd+wu on gpsimd
        SWDGE -- triggers issue in parallel across three sequencers.
  warm: 4 dummy matmuls at block entry (no memset gate) keep PE busy so
        HAM un-throttles before/early-into the real work.
  down: ph[h,c] += wd[k,h].T @ xT[k,c], DoubleRow fp8, per ko-pair the
        N=512 main tile and N=32 tail tile share one LDWEIGHTS.
  silu: scalar engine, chunks [0:128],[128:512],[512:544], fp8 out.
  up:   py[c,n] = h[h,cb].T @ wu[h,n]; 16 MMs rotate 4 PSUM slots of
        [128,1024]; tail rows go through 4 col-tiled (tile_position)
        MMs packing [32rows x 2048] as [128part x 512] in one slot.
  cast: PSUM->SBUF fp8; Vector owns banks 2-3/6-7, Scalar owns 0-1/4-5
        (never a same-bank conflict); out DMAs per row-block as soon as
        both halves are cast (gpsimd: cb0/cb1, sync: cb2/cb3,
        scalar: tail).
"""

import numpy as np
import ml_dtypes

N_TASKS = 8
SIZE = 2048
HID = 128
P = 128
KD = SIZE // P           # 16 contraction chunks for the down projection
CAP = 544                # per-core routed-row capacity (max seed-0 count is 527)
R = CAP - 512            # tail rows handled via partition-packed up matmuls
F0 = 512                 # down main col-tile
WSCALE = 16.0            # host pre-scale on Wd/Wu for fp8 dynamic range
ACT_FUNC = "Silu"

_NC = None


def _build_nc():
    import concourse.mybir as mybir
    from concourse import bacc

    dt = mybir.dt
    f8 = dt.float8e4
    act_fn = getattr(mybir.ActivationFunctionType, ACT_FUNC)
    import concourse.bass as cbass

    # The constructor tail emits a full all-engine EVSEM barrier (~3.5us on
    # silicon) guarding preamble state this kernel never reads. Every
    # cross-engine dependency below is explicitly semaphore-gated, so skip
    # the entry barrier; Block exit still emits its own.
    _orig_barrier = cbass.Bass.all_engine_barrier
    cbass.Bass.all_engine_barrier = lambda self, **kw: None
    try:
        nc = bacc.Bacc(
            "TRN2", debug=False, num_devices=N_TASKS, monotonic_sem_count=0
        )
    finally:
        cbass.Bass.all_engine_barrier = _orig_barrier

    xt = nc.dram_tensor("xt", [P, KD * CAP], f8, kind="ExternalInput")
    wdp = nc.dram_tensor("wdp", [P, KD * P], f8, kind="ExternalInput")
    wu = nc.dram_tensor("wu", [P, SIZE], f8, kind="ExternalInput")
    bdp = nc.dram_tensor("bdp", [P, 1], dt.float32, kind="ExternalInput")
    out = nc.dram_tensor("out", [512, SIZE], f8, kind="ExternalOutput")
    outt = nc.dram_tensor("outt", [P, 512], f8, kind="ExternalOutput")

    wd_sb = nc.alloc_sbuf_tensor("wd_sb", [P, KD, P], f8).ap()
    x_sb = nc.alloc_sbuf_tensor("x_sb", [P, KD, CAP], f8).ap()
    wu_sb = nc.alloc_sbuf_tensor("wu_sb", [P, SIZE], f8).ap()
    bd_sb = nc.alloc_sbuf_tensor("bd_sb", [P, 1], dt.float32).ap()
    h_sb = nc.alloc_sbuf_tensor("h_sb", [P, CAP], f8).ap()
    o_sb = nc.alloc_sbuf_tensor("o_sb", [P, 4, SIZE], f8).ap()
    ot_sb = nc.alloc_sbuf_tensor("ot_sb", [P, 512], f8).ap()
    dum_sb = nc.alloc_sbuf_tensor("dum_sb", [P, F0], f8).ap()
    dsc_sb = nc.alloc_sbuf_tensor("dsc_sb", [P, 1], dt.float32).ap()

    # All 8 PSUM banks as one tensor; 512-col bank-aligned slices.
    # S0 = cols 0:1024 (banks 0-1, down ph lives here first),
    # S1 = 1024:2048, S2 = 2048:3072, S3 = 3072:4096.
    pall = nc.alloc_psum_tensor("pall", [P, 4096], dt.float32).ap()
    SLOT = [0, 1024, 2048, 3072]
    ph0 = pall[:, 0:F0]          # down main accumulator (bank 0)
    ph1 = pall[:, F0 : F0 + R]   # down tail accumulator (bank 1)

    sX = [nc.alloc_semaphore(f"sX{q}") for q in range(4)]
    sWd = nc.alloc_semaphore("sWd")
    sWu = nc.alloc_semaphore("sWu")
    sBd = nc.alloc_semaphore("sBd")
    sDN = nc.alloc_semaphore("sDN")
    sSil = nc.alloc_semaphore("sSil")
    sUP = nc.alloc_semaphore("sUP")
    sCV = nc.alloc_semaphore("sCV")
    sCS = nc.alloc_semaphore("sCS")
    sOUT = nc.alloc_semaphore("sOUT")
    sOUTg = nc.alloc_semaphore("sOUTg")
    sOUTs = nc.alloc_semaphore("sOUTs")

    # up matmul g (0..15): pair p = g//2, cb = g//4, slot rotation
    # S1,S2,S3,S0 per pair; n-chunk = g%4 within cb? No: g covers cb's
    # n-chunks in order: cb = g//4, ncx = g%4; pair p groups (ncx 0,1)
    # and (ncx 2,3) -> slot index (p+1)%4.
    def up_slot(p):
        return SLOT[(p + 1) % 4]

    with nc.Block(no_gpsimd_drain=True) as block:

        @block.sync
        def _(sync):
            xv = xt.ap().rearrange("p (ko c) -> p ko c", c=CAP)
            for q in range(4):
                sync.dma_start(
                    x_sb[:, 4 * q : 4 * (q + 1)], xv[:, 4 * q : 4 * (q + 1)]
                ).then_inc(sX[q], 16)
            for cb in (2, 3):
                sync.wait_ge(sCV, cb + 1)
                sync.wait_ge(sCS, cb + 1)
                sync.dma_start(
                    out.ap()[cb * P : (cb + 1) * P, :], o_sb[:, cb, :]
                ).then_inc(sOUT, 16)
            sync.wait_ge(sOUT, 32)
            sync.wait_ge(sOUTg, 32)
            sync.wait_ge(sOUTs, 16)

        @block.gpsimd
        def _(gpsimd):
            gpsimd.dma_start(bd_sb, bdp.ap()).then_inc(sBd, 16)
            gpsimd.dma_start(wu_sb, wu.ap()).then_inc(sWu, 16)
            for cb in (0, 1):
                gpsimd.wait_ge(sCV, cb + 1)
                gpsimd.wait_ge(sCS, cb + 1)
                gpsimd.dma_start(
                    out.ap()[cb * P : (cb + 1) * P, :], o_sb[:, cb, :]
                ).then_inc(sOUTg, 16)

        @block.tensor
        def _(tensor):
            # HAM warmup on uninitialized data while the input DMAs land;
            # every later PSUM write uses start=True so garbage never leaks.
            for _ in range(4):
                tensor.matmul(
                    pall[:, SLOT[3] : SLOT[3] + F0],
                    dum_sb[:, :P],
                    dum_sb[:, :F0],
                    start=True,
                    stop=True,
                )
            DR = mybir.MatmulPerfMode.DoubleRow
            tensor.wait_ge(sWd, 16)
            for j in range(8):  # ko pairs
                if j % 2 == 0:
                    tensor.wait_ge(sX[j // 2], 16)
                ko = 2 * j
                last = j == 7
                tensor.matmul(
                    ph0,
                    wd_sb[:, ko : ko + 2, :],
                    x_sb[:, ko : ko + 2, 0:F0],
                    start=(j == 0),
                    stop=last,
                    perf_mode=DR,
                )
                mm = tensor.matmul(
                    ph1,
                    wd_sb[:, ko : ko + 2, :],
                    x_sb[:, ko : ko + 2, F0:CAP],
                    start=(j == 0),
                    stop=last,
                    perf_mode=DR,
                )
            mm.then_inc(sDN, 1)
            # up: 16 main MMs, h block stationary (shared across a cb's 4
            # n-chunks), slots rotate S1,S2,S3,S0; casts gate slot reuse.
            tensor.wait_ge(sWu, 16)
            for g in range(16):
                cb, ncx = divmod(g, 4)
                p = g // 2
                if g == 0:
                    tensor.wait_ge(sSil, 1)
                elif g == 4:
                    tensor.wait_ge(sSil, 2)
                elif g == 6:
                    tensor.wait_ge(sSil, 3)  # S0 overlaps down ph region
                elif g == 8:
                    tensor.wait_ge(sCV, 1)   # slot S1 cast done
                elif g == 10:
                    tensor.wait_ge(sCS, 1)   # S2
                elif g == 12:
                    tensor.wait_ge(sCV, 2)   # S3
                elif g == 14:
                    tensor.wait_ge(sCS, 2)   # S0
                base = up_slot(p) + (g % 2) * 512
                tensor.matmul(
                    pall[:, base : base + 512],
                    h_sb[:, cb * P : (cb + 1) * P],
                    wu_sb[:, ncx * 512 : (ncx + 1) * 512],
                    start=True,
                    stop=True,
                ).then_inc(sUP, 1)
            # tail rows: 4 col-tiled MMs pack [R x 2048] into S1[:, :512]
            # as [4*32 partitions x 512]; n-chunk j lands at partitions 32j.
            tensor.wait_ge(sCV, 3)  # S1's second cast done
            for j in range(4):
                tensor.matmul(
                    pall[32 * j : 32 * (j + 1), SLOT[1] : SLOT[1] + 512],
                    h_sb[:, F0:CAP],
                    wu_sb[:, j * 512 : (j + 1) * 512],
                    start=True,
                    stop=True,
                    tile_position=(0, 32 * j),
                ).then_inc(sUP, 1)

        @block.scalar
        def _(scalar):
            scalar.dma_start(
                wd_sb, wdp.ap().rearrange("p (ko m) -> p ko m", m=P)
            ).then_inc(sWd, 16)
            # dummy silu first: loads silu_and_others (which contains copy)
            # during the DMA window -- one table load for the whole kernel
            scalar.activation(dsc_sb, dum_sb[:, :1], act_fn)
            scalar.wait_ge(sBd, 16)
            scalar.wait_ge(sDN, 1)
            scalar.activation(
                h_sb[:, 0:P], pall[:, 0:P], act_fn, bias=bd_sb, scale=1.0 / WSCALE
            ).then_inc(sSil, 1)
            scalar.activation(
                h_sb[:, P:F0], pall[:, P:F0], act_fn, bias=bd_sb, scale=1.0 / WSCALE
            ).then_inc(sSil, 1)
            scalar.activation(
                h_sb[:, F0:CAP],
                pall[:, F0 : F0 + R],
                act_fn,
                bias=bd_sb,
                scale=1.0 / WSCALE,
            ).then_inc(sSil, 1)
            # scalar casts: slots S2 (banks 4-5) and S0 (banks 0-1) only
            for i, (slot, cb) in enumerate(
                [(SLOT[2], 0), (SLOT[0], 1), (SLOT[2], 2), (SLOT[0], 3)]
            ):
                scalar.wait_ge(sUP, 4 * (i + 1))
                scalar.copy(
                    o_sb[:, cb, 1024:2048], pall[:, slot : slot + 1024]
                ).then_inc(sCS, 1)
            scalar.wait_ge(sCV, 5)
            scalar.dma_start(outt.ap(), ot_sb).then_inc(sOUTs, 16)

        @block.vector
        def _(vector):
            # vector casts: slots S1 (banks 2-3) and S3 (banks 6-7) only
            for i, (slot, cb) in enumerate(
                [(SLOT[1], 0), (SLOT[3], 1), (SLOT[1], 2), (SLOT[3], 3)]
            ):
                vector.wait_ge(sUP, 2 + 4 * i)
                vector.tensor_copy(
                    o_sb[:, cb, 0:1024], pall[:, slot : slot + 1024]
                ).then_inc(sCV, 1)
            vector.wait_ge(sUP, 20)
            vector.tensor_copy(
                ot_sb, pall[:, SLOT[1] : SLOT[1] + 512]
            ).then_inc(sCV, 1)

    nc.compile()
    return nc


def _get_nc():
    global _NC
    if _NC is None:
        _NC = _build_nc()
    return _NC


def kernel(x, Wd, bd, Wu, bu, task_id):
    from concourse.bass_utils import run_bass_kernel_spmd

    x = np.asarray(x, dtype=np.float32)
    Wd = np.asarray(Wd, dtype=np.float32)
    bd = np.asarray(bd, dtype=np.float32)
    Wu = np.asarray(Wu, dtype=np.float32)
    bu = np.asarray(bu, dtype=np.float32)
    tid = np.asarray(task_id).astype(np.int64)

    f8 = ml_dtypes.float8_e4m3
    valid = tid >= 0
    t_clip = np.clip(tid, 0, N_TASKS - 1)

    in_maps = []
    rows_per_task = []
    overflow = []  # (task, rows) beyond CAP -> host fallback, keeps correctness
    for t in range(N_TASKS):
        rows = np.nonzero(valid & (t_clip == t))[0]
        if rows.size > CAP:
            overflow.append((t, rows[CAP:]))
            rows = rows[:CAP]
        rows_per_task.append(rows)

        xr = np.zeros((CAP, SIZE), dtype=np.float32)
        xr[: rows.size] = x[rows]
        xtp = xr.reshape(CAP, KD, P).transpose(2, 1, 0).reshape(P, KD * CAP)
        wdpk = (
            (Wd[t] * WSCALE).reshape(KD, P, P).transpose(1, 0, 2).reshape(P, KD * P)
        )
        in_maps.append(
            {
                "xt": np.ascontiguousarray(xtp).astype(f8),
                "wdp": np.ascontiguousarray(wdpk).astype(f8),
                "wu": (Wu[t] * WSCALE).astype(f8),
                "bdp": np.ascontiguousarray(bd[t].reshape(P, 1)),
            }
        )

    global _last_in_maps
    _last_in_maps = in_maps
    nc = _get_nc()
    res = run_bass_kernel_spmd(nc, in_maps, list(range(N_TASKS))).results

    out = x.copy()
    for t in range(N_TASKS):
        rows = rows_per_task[t]
        if rows.size == 0:
            continue
        o = np.asarray(res[t]["out"]).astype(np.float32)  # [512, SIZE]
        ot = np.asarray(res[t]["outt"]).astype(np.float32)  # [128, 512]
        tail = ot.reshape(4, 32, 512).transpose(1, 0, 2).reshape(R, SIZE)
        full = np.concatenate([o, tail], axis=0)
        delta = full[: rows.size] * (1.0 / WSCALE)
        out[rows] += delta + bu[t][None, :]
    for t, rows in overflow:
        hz = x[rows] @ Wd[t] + bd[t]
        h = hz / (1.0 + np.exp(-hz))
        out[rows] += h @ Wu[t] + bu[t]
    return out


# revision 7
# speedup vs baseline: 1.0542x; 1.0542x over previous
"""Per-task adapter (MoE routing) on 8 TRN2 NeuronCores.

Strategy: expert-parallel. Host routes rows by task_id so core t gets all
rows with task t, each core computes only its own expert's adapter delta
= silu(x @ Wd[t] + bd[t]) @ Wu[t], and the host scatters deltas back,
adding the f32 residual x and bu[t].

Device kernel is raw bacc (no TileContext) with hand-placed semaphores,
fp8-e4m3 I/O (weights pre-scaled by 16 on the host; the 1/16 is folded
into the silu activation scale, and the up output is descaled on host).

Dataflow per core (capacity CAP=544 = 512 + 32 tail rows):
  in:   x quarters on sync HWDGE, wd on scalar HWDGE, bd+wu on gpsimd
        SWDGE -- triggers issue in parallel across three sequencers.
  warm: 4 dummy matmuls at block entry (no memset gate) keep PE busy so
        HAM un-throttles before/early-into the real work.
  down: ph[h,c] += wd[k,h].T @ xT[k,c], DoubleRow fp8, per ko-pair the
        N=512 main tile and N=32 tail tile share one LDWEIGHTS.
  silu: scalar engine, chunks [0:128],[128:512],[512:544], fp8 out.
  up:   py[c,n] = h[h,cb].T @ wu[h,n]; 16 MMs rotate 4 PSUM slots of
        [128,1024]; tail rows go through 4 col-tiled (tile_position)
        MMs packing [32rows x 2048] as [128part x 512] in one slot.
  cast: PSUM->SBUF fp8; Vector owns banks 2-3/6-7, Scalar owns 0-1/4-5
        (never a same-bank conflict); out DMAs per row-block as soon as
        both halves are cast (gpsimd: cb0/cb1, sync: cb2/cb3,
        scalar: tail).
"""

import numpy as np
import ml_dtypes

N_TASKS = 8
SIZE = 2048
HID = 128
P = 128
KD = SIZE // P           # 16 contraction chunks for the down projection
CAP = 544                # per-core routed-row capacity (max seed-0 count is 527)
R = CAP - 512            # tail rows handled via partition-packed up matmuls
F0 = 512                 # down main col-tile
WSCALE = 16.0            # host pre-scale on Wd/Wu for fp8 dynamic range
ACT_FUNC = "Silu"

_NC = None


def _build_nc():
    import concourse.mybir as mybir
    from concourse import bacc

    dt = mybir.dt
    f8 = dt.float8e4
    act_fn = getattr(mybir.ActivationFunctionType, ACT_FUNC)
    import concourse.bass as cbass

    # The constructor tail emits a full all-engine EVSEM barrier (~3.5us on
    # silicon) guarding preamble state this kernel never reads. Every
    # cross-engine dependency below is explicitly semaphore-gated, so skip
    # the entry barrier; Block exit still emits its own.
    _orig_barrier = cbass.Bass.all_engine_barrier
    cbass.Bass.all_engine_barrier = lambda self, **kw: None
    try:
        nc = bacc.Bacc(
            "TRN2", debug=False, num_devices=N_TASKS, monotonic_sem_count=0
        )
    finally:
        cbass.Bass.all_engine_barrier = _orig_barrier

    xt = nc.dram_tensor("xt", [P, KD * CAP], f8, kind="ExternalInput")
    wdp = nc.dram_tensor("wdp", [P, KD * P], f8, kind="ExternalInput")
    wu = nc.dram_tensor("wu", [P, SIZE], f8, kind="ExternalInput")
    bdp = nc.dram_tensor("bdp", [P, 1], dt.float32, kind="ExternalInput")
    out = nc.dram_tensor("out", [512, SIZE], f8, kind="ExternalOutput")
    outt = nc.dram_tensor("outt", [P, 512], f8, kind="ExternalOutput")

    wd_sb = nc.alloc_sbuf_tensor("wd_sb", [P, KD, P], f8).ap()
    x_sb = nc.alloc_sbuf_tensor("x_sb", [P, KD, CAP], f8).ap()
    wu_sb = nc.alloc_sbuf_tensor("wu_sb", [P, SIZE], f8).ap()
    bd_sb = nc.alloc_sbuf_tensor("bd_sb", [P, 1], dt.float32).ap()
    h_sb = nc.alloc_sbuf_tensor("h_sb", [P, CAP], f8).ap()
    o_sb = nc.alloc_sbuf_tensor("o_sb", [P, 4, SIZE], f8).ap()
    ot_sb = nc.alloc_sbuf_tensor("ot_sb", [P, 512], f8).ap()
    dum_sb = nc.alloc_sbuf_tensor("dum_sb", [P, F0], f8).ap()
    dsc_sb = nc.alloc_sbuf_tensor("dsc_sb", [P, 1], dt.float32).ap()

    # All 8 PSUM banks as one tensor; 512-col bank-aligned slices.
    # S0 = cols 0:1024 (banks 0-1, down ph lives here first),
    # S1 = 1024:2048, S2 = 2048:3072, S3 = 3072:4096.
    pall = nc.alloc_psum_tensor("pall", [P, 4096], dt.float32).ap()
    SLOT = [0, 1024, 2048, 3072]
    ph0 = pall[:, 0:F0]          # down main accumulator (bank 0)
    ph1 = pall[:, F0 : F0 + R]   # down tail accumulator (bank 1)

    sX = [nc.alloc_semaphore(f"sX{q}") for q in range(4)]
    sWd = nc.alloc_semaphore("sWd")
    sWu = nc.alloc_semaphore("sWu")
    sBd = nc.alloc_semaphore("sBd")
    sDN = nc.alloc_semaphore("sDN")
    sSil = nc.alloc_semaphore("sSil")
    sUP = nc.alloc_semaphore("sUP")
    sCV = nc.alloc_semaphore("sCV")
    sCS = nc.alloc_semaphore("sCS")
    sOUT = nc.alloc_semaphore("sOUT")
    sOUTg = nc.alloc_semaphore("sOUTg")
    sOUTs = nc.alloc_semaphore("sOUTs")

    # up matmul g (0..15): pair p = g//2, cb = g//4, slot rotation
    # S1,S2,S3,S0 per pair; n-chunk = g%4 within cb? No: g covers cb's
    # n-chunks in order: cb = g//4, ncx = g%4; pair p groups (ncx 0,1)
    # and (ncx 2,3) -> slot index (p+1)%4.
    def up_slot(p):
        return SLOT[(p + 1) % 4]

    with nc.Block(no_gpsimd_drain=True) as block:

        @block.sync
        def _(sync):
            xv = xt.ap().rearrange("p (ko c) -> p ko c", c=CAP)
            for q in range(4):
                sync.dma_start(
                    x_sb[:, 4 * q : 4 * (q + 1)], xv[:, 4 * q : 4 * (q + 1)]
                ).then_inc(sX[q], 16)
            for cb in (2, 3):
                sync.wait_ge(sCV, cb + 1)
                sync.wait_ge(sCS, cb + 1)
                sync.dma_start(
                    out.ap()[cb * P : (cb + 1) * P, :], o_sb[:, cb, :]
                ).then_inc(sOUT, 16)
            sync.wait_ge(sOUT, 32)
            sync.wait_ge(sOUTg, 32)
            sync.wait_ge(sOUTs, 16)

        @block.gpsimd
        def _(gpsimd):
            for cb in (0, 1):
                gpsimd.wait_ge(sCV, cb + 1)
                gpsimd.wait_ge(sCS, cb + 1)
                gpsimd.dma_start(
                    out.ap()[cb * P : (cb + 1) * P, :], o_sb[:, cb, :]
                ).then_inc(sOUTg, 16)

        @block.tensor
        def _(tensor):
            # HAM warmup on uninitialized data while the input DMAs land;
            # every later PSUM write uses start=True so garbage never leaks.
            for _ in range(4):
                tensor.matmul(
                    pall[:, SLOT[3] : SLOT[3] + F0],
                    dum_sb[:, :P],
                    dum_sb[:, :F0],
                    start=True,
                    stop=True,
                )
            DR = mybir.MatmulPerfMode.DoubleRow
            tensor.wait_ge(sWd, 16)
            for j in range(8):  # ko pairs
                if j % 2 == 0:
                    tensor.wait_ge(sX[j // 2], 16)
                ko = 2 * j
                last = j == 7
                tensor.matmul(
                    ph0,
                    wd_sb[:, ko : ko + 2, :],
                    x_sb[:, ko : ko + 2, 0:F0],
                    start=(j == 0),
                    stop=last,
                    perf_mode=DR,
                )
                mm = tensor.matmul(
                    ph1,
                    wd_sb[:, ko : ko + 2, :],
                    x_sb[:, ko : ko + 2, F0:CAP],
                    start=(j == 0),
                    stop=last,
                    perf_mode=DR,
                )
            mm.then_inc(sDN, 1)
            # up: 16 main MMs, h block stationary (shared across a cb's 4
            # n-chunks), slots rotate S1,S2,S3,S0; casts gate slot reuse.
            tensor.wait_ge(sWu, 16)
            for g in range(16):
                cb, ncx = divmod(g, 4)
                p = g // 2
                if g == 0:
                    tensor.wait_ge(sSil, 1)
                elif g == 4:
                    tensor.wait_ge(sSil, 2)
                elif g == 6:
                    tensor.wait_ge(sSil, 3)  # S0 overlaps down ph region
                elif g == 8:
                    tensor.wait_ge(sCV, 1)   # slot S1 cast done
                elif g == 10:
                    tensor.wait_ge(sCS, 1)   # S2
                elif g == 12:
                    tensor.wait_ge(sCV, 2)   # S3
                elif g == 14:
                    tensor.wait_ge(sCS, 2)   # S0
                base = up_slot(p) + (g % 2) * 512
                tensor.matmul(
                    pall[:, base : base + 512],
                    h_sb[:, cb * P : (cb + 1) * P],
                    wu_sb[:, ncx * 512 : (ncx + 1) * 512],
                    start=True,
                    stop=True,
                ).then_inc(sUP, 1)
            # tail rows: 4 col-tiled MMs pack [R x 2048] into S1[:, :512]
            # as [4*32 partitions x 512]; n-chunk j lands at partitions 32j.
            tensor.wait_ge(sCV, 3)  # S1's second cast done
            for j in range(4):
                tensor.matmul(
                    pall[32 * j : 32 * (j + 1), SLOT[1] : SLOT[1] + 512],
                    h_sb[:, F0:CAP],
                    wu_sb[:, j * 512 : (j + 1) * 512],
                    start=True,
                    stop=True,
                    tile_position=(0, 32 * j),
                ).then_inc(sUP, 1)

        @block.scalar
        def _(scalar):
            scalar.dma_start(
                wd_sb, wdp.ap().rearrange("p (ko m) -> p ko m", m=P)
            ).then_inc(sWd, 16)
            # wu/bd ride the scalar HWDGE queue too: SWDGE (gpsimd) packet
            # drain is slow enough to starve the x quarters' completion sems
            scalar.dma_start(wu_sb, wu.ap()).then_inc(sWu, 16)
            scalar.dma_start(bd_sb, bdp.ap()).then_inc(sBd, 16)
            # dummy silu first: loads silu_and_others (which contains copy)
            # during the DMA window -- one table load for the whole kernel
            scalar.activation(dsc_sb, dum_sb[:, :1], act_fn)
            scalar.wait_ge(sBd, 16)
            scalar.wait_ge(sDN, 1)
            scalar.activation(
                h_sb[:, 0:P], pall[:, 0:P], act_fn, bias=bd_sb, scale=1.0 / WSCALE
            ).then_inc(sSil, 1)
            scalar.activation(
                h_sb[:, P:F0], pall[:, P:F0], act_fn, bias=bd_sb, scale=1.0 / WSCALE
            ).then_inc(sSil, 1)
            scalar.activation(
                h_sb[:, F0:CAP],
                pall[:, F0 : F0 + R],
                act_fn,
                bias=bd_sb,
                scale=1.0 / WSCALE,
            ).then_inc(sSil, 1)
            # scalar casts: slots S2 (banks 4-5) and S0 (banks 0-1) only
            for i, (slot, cb) in enumerate(
                [(SLOT[2], 0), (SLOT[0], 1), (SLOT[2], 2), (SLOT[0], 3)]
            ):
                scalar.wait_ge(sUP, 4 * (i + 1))
                scalar.copy(
                    o_sb[:, cb, 1024:2048], pall[:, slot : slot + 1024]
                ).then_inc(sCS, 1)
            scalar.wait_ge(sCV, 5)
            scalar.dma_start(outt.ap(), ot_sb).then_inc(sOUTs, 16)

        @block.vector
        def _(vector):
            # vector casts: slots S1 (banks 2-3) and S3 (banks 6-7) only
            for i, (slot, cb) in enumerate(
                [(SLOT[1], 0), (SLOT[3], 1), (SLOT[1], 2), (SLOT[3], 3)]
            ):
                vector.wait_ge(sUP, 2 + 4 * i)
                vector.tensor_copy(
                    o_sb[:, cb, 0:1024], pall[:, slot : slot + 1024]
                ).then_inc(sCV, 1)
            vector.wait_ge(sUP, 20)
            vector.tensor_copy(
                ot_sb, pall[:, SLOT[1] : SLOT[1] + 512]
            ).then_inc(sCV, 1)

    nc.compile()
    return nc


def _get_nc():
    global _NC
    if _NC is None:
        _NC = _build_nc()
    return _NC


def kernel(x, Wd, bd, Wu, bu, task_id):
    from concourse.bass_utils import run_bass_kernel_spmd

    x = np.asarray(x, dtype=np.float32)
    Wd = np.asarray(Wd, dtype=np.float32)
    bd = np.asarray(bd, dtype=np.float32)
    Wu = np.asarray(Wu, dtype=np.float32)
    bu = np.asarray(bu, dtype=np.float32)
    tid = np.asarray(task_id).astype(np.int64)

    f8 = ml_dtypes.float8_e4m3
    valid = tid >= 0
    t_clip = np.clip(tid, 0, N_TASKS - 1)

    in_maps = []
    rows_per_task = []
    overflow = []  # (task, rows) beyond CAP -> host fallback, keeps correctness
    for t in range(N_TASKS):
        rows = np.nonzero(valid & (t_clip == t))[0]
        if rows.size > CAP:
            overflow.append((t, rows[CAP:]))
            rows = rows[:CAP]
        rows_per_task.append(rows)

        xr = np.zeros((CAP, SIZE), dtype=np.float32)
        xr[: rows.size] = x[rows]
        xtp = xr.reshape(CAP, KD, P).transpose(2, 1, 0).reshape(P, KD * CAP)
        wdpk = (
            (Wd[t] * WSCALE).reshape(KD, P, P).transpose(1, 0, 2).reshape(P, KD * P)
        )
        in_maps.append(
            {
                "xt": np.ascontiguousarray(xtp).astype(f8),
                "wdp": np.ascontiguousarray(wdpk).astype(f8),
                "wu": (Wu[t] * WSCALE).astype(f8),
                "bdp": np.ascontiguousarray(bd[t].reshape(P, 1)),
            }
        )

    global _last_in_maps
    _last_in_maps = in_maps
    nc = _get_nc()
    res = run_bass_kernel_spmd(nc, in_maps, list(range(N_TASKS))).results

    out = x.copy()
    for t in range(N_TASKS):
        rows = rows_per_task[t]
        if rows.size == 0:
            continue
        o = np.asarray(res[t]["out"]).astype(np.float32)  # [512, SIZE]
        ot = np.asarray(res[t]["outt"]).astype(np.float32)  # [128, 512]
        tail = ot.reshape(4, 32, 512).transpose(1, 0, 2).reshape(R, SIZE)
        full = np.concatenate([o, tail], axis=0)
        delta = full[: rows.size] * (1.0 / WSCALE)
        out[rows] += delta + bu[t][None, :]
    for t, rows in overflow:
        hz = x[rows] @ Wd[t] + bd[t]
        h = hz / (1.0 + np.exp(-hz))
        out[rows] += h @ Wu[t] + bu[t]
    return out
